# revision 34
# baseline (speedup 1.0000x reference)
"""DCRNN (PEMS-BAY) Trainium2 Bass kernel, data-parallel over batch on 8 cores.

Transpose-free gconv via S^2 precompute, fp16 matmuls/states, fp32 psum
for the diffusion accumulations, fp16 psum staging for the W-products.

Layouts per core (local batch BL=8, split in 2 halves of HB=4):
  A1: [feature partitions, b*NB + n]   (state tiles XH/XR: rows 0:64 = h|rh,
      rows 64:64+din = x)
  B:  [node-chunk partitions (128/128/69), b*F + f]  (W-product tiles)
gconv:  pre = X@A0 + S@(X@W1) + S^2@(X@(2*W2))      [A0 = W0 - W2]
  ph1/ph3 W-products: ONE matmul per (b, chunk) with packed rhs [W1 | 2*W2]
  into fp16 psum, ONE evac copy per chunk into a packed B-tile.
  ph2/ph4: diffusion accumulation per batch in fp32 psum 2-bank pair tiles
  (P0 = a0 start, then S@P1 + S2@P2 with S/S2 as rhs, lhsT = B-tile slices),
  activations applied per PAIR via 3-dim APs spanning both banks.
GRU elementwise merged to [64, HAF] half-wide ops; mirror copies via DMA.
"""
import sys
import os
import numpy as np

sys.path.insert(0, "/opt/trn_rl_repo")

import concourse.bass as bass  # noqa: E402
import concourse.mybir as mybir  # noqa: E402
import concourse.tile as tile  # noqa: E402
from concourse import bacc  # noqa: E402
from concourse.bass_utils import run_bass_kernel_spmd  # noqa: E402

# problem constants
N = 325
B = 64
T = 12
HZ = 12
U = 64
DIN = 2
DOUT = 1
NCORES = 8
BL = B // NCORES          # 8 local batch
NB = 328                  # padded node stride per batch (16B aligned)
AF = BL * NB              # 2624 A-layout free width
NCH = [(0, 128), (128, 128), (256, 69)]   # node chunks (offset, len)
HB = BL // 2              # 4 batches per half
HAF = HB * NB             # 1312 A cols per half

F32 = mybir.dt.float32
F16 = mybir.dt.float16
AFT = mybir.ActivationFunctionType
ALU = mybir.AluOpType

CELLS = ["enc0", "enc1", "dec0", "dec1"]
CELL_DIN = {"enc0": DIN, "enc1": U, "dec0": DOUT, "dec1": U}

_BUILD_CACHE = {}
LAST_RESULT = None


def _install_ntff_hook():
    """Register the axon NTFF profiling hook if the image lacks antenv.axon_hooks."""
    import types
    import antenv
    if getattr(antenv, "axon_hooks", None) is not None:
        return
    m = types.ModuleType("antenv.axon_hooks")
    state = {"h": None}
    m.set_axon_ntff_profile_hook = lambda h: state.__setitem__("h", h)
    m.get_axon_ntff_profile_hook = lambda: state["h"]
    sys.modules["antenv.axon_hooks"] = m
    antenv.axon_hooks = m
    try:
        from trn_agent_boot.trn_boot import _ntff_profile_via_ctypes
        hook = _ntff_profile_via_ctypes("/opt/axon/libaxon_pjrt.so")
        if hook is not None:
            m.set_axon_ntff_profile_hook(hook)
    except Exception:
        pass


def _pad_w(w, din, fout):
    """(3F, fout) -> three [128, fout] fp32 mats A0, W1, 2*W2.

    Padded row map: rows 0:64 <- h/rh features (orig rows din:F),
    rows 64:64+din <- x features (orig rows 0:din). Others zero.
    """
    f = din + U
    w0, w1, w2 = w[0:f], w[f:2 * f], w[2 * f:3 * f]

    def pad(m):
        p = np.zeros((128, fout), np.float32)
        p[0:64] = m[din:f]
        p[64:64 + din] = m[0:din]
        return p

    return pad(w0 - w2), pad(w1), pad(2.0 * w2)


def _build(nsteps_enc, nsteps_dec):
    key = (nsteps_enc, nsteps_dec)
    if key in _BUILD_CACHE:
        return _BUILD_CACHE[key]

    nc = bacc.Bacc()
    # ---- DRAM params ----
    # all fp16 weights live in one blob (single DMA at startup):
    # per cell [gA0 128 | gW12 256 | cA0 64 | cW12 128] = 576 cols,
    # then s chunks 3x325, s2 chunks 3x325, pW 1 col.
    NWB = 4 * 576 + 6 * N + 1
    x_in = nc.declare_dram_parameter("x", [T, DIN, AF], F16, isOutput=False)
    wb_in = nc.declare_dram_parameter("wb", [128, NWB], F16, isOutput=False)
    bb_in = nc.declare_dram_parameter("bb", [128, 9], F32, isOutput=False)
    out_d = nc.declare_dram_parameter("out", [HZ, 1, BL, N], F16, isOutput=True)

    with tile.TileContext(nc) as tc:
        with tc.tile_pool(name="const", bufs=1) as cp, \
             tc.tile_pool(name="state", bufs=1) as st, \
             tc.tile_pool(name="bprod", bufs=1) as bp, \
             tc.tile_pool(name="pstage", bufs=4, space="PSUM") as psp, \
             tc.tile_pool(name="pacc", bufs=2, space="PSUM") as pap:

            # ---- constants to SBUF (one DMA each for wb/bb) ----
            wbt = cp.tile([128, NWB], F16, tag="wb", name="wb")
            bbt = cp.tile([128, 9], F32, tag="bb", name="bb")
            nc.sync.dma_start(out=wbt, in_=wb_in[:])
            nc.sync.dma_start(out=bbt, in_=bb_in[:])
            wt = {}
            for i, c in enumerate(CELLS):
                o = i * 576
                wt[f"{c}_gA0"] = wbt[0:128, o:o + 128]
                wt[f"{c}_gW12"] = wbt[0:128, o + 128:o + 384]
                wt[f"{c}_cA0"] = wbt[0:128, o + 384:o + 448]
                wt[f"{c}_cW12"] = wbt[0:128, o + 448:o + 576]
                wt[f"{c}_gb"] = bbt[0:128, 2 * i:2 * i + 1]
                wt[f"{c}_cb2"] = [bbt[0:64, 2 * i + 1:2 * i + 2],
                                  bbt[64:128, 2 * i + 1:2 * i + 2]]
            so = 4 * 576
            s_t = [wbt[0:cl, so + ci * N: so + (ci + 1) * N]
                   for ci, (c0, cl) in enumerate(NCH)]
            s2_t = [wbt[0:cl, so + (3 + ci) * N: so + (4 + ci) * N]
                    for ci, (c0, cl) in enumerate(NCH)]
            wt["pW"] = wbt[0:64, NWB - 1:NWB]
            wt["pb1"] = bbt[0:1, 8:9]

            # ---- state tiles ----
            xh, xr = {}, {}
            ru, ct, uu = {}, {}, {}
            for hf in range(2):
                for c in CELLS:
                    xh[(c, hf)] = st.tile([128, HAF], F16, tag=f"xh_{c}_{hf}",
                                          name=f"xh_{c}_{hf}")
                    xr[(c, hf)] = st.tile([128, HAF], F16, tag=f"xr_{c}_{hf}",
                                          name=f"xr_{c}_{hf}")
                for lv in range(2):
                    ru[(hf, lv)] = st.tile([128, HAF], F16, tag=f"ru{hf}{lv}",
                                           name=f"ru{hf}{lv}")
                    ct[(hf, lv)] = st.tile([64, HAF], F16, tag=f"ct{hf}{lv}",
                                           name=f"ct{hf}{lv}")
                    uu[(hf, lv)] = st.tile([64, HAF], F16, tag=f"uu{hf}{lv}",
                                           name=f"uu{hf}{lv}")

            # Only xh/xr need zeroing (their unwritten rows feed matmul
            # contractions where a NaN would poison the psum even against
            # zero weights). ru/ct/uu pad garbage stays confined to pad
            # columns which nothing downstream reads. Ordered by first use.
            ms = []
            for c in CELLS:
                for hf in range(2):
                    ms += [xh[(c, hf)], xr[(c, hf)]]
            for tl in ms:
                nc.vector.memset(tl[:, :], 0.0)

            def cell_phases(cname, hf, mirror_to=None):
                """One DCGRU cell on one batch-half, as 4 phase thunks.

                mirror_to: layer name whose x rows receive h' per batch.
                """
                xh_t = xh[(cname, hf)]
                xr_t = xr[(cname, hf)]
                gb = wt[f"{cname}_gb"]
                cb2 = wt[f"{cname}_cb2"]
                ga0 = wt[f"{cname}_gA0"]
                gw12 = wt[f"{cname}_gW12"]
                ca0 = wt[f"{cname}_cA0"]
                cw12 = wt[f"{cname}_cW12"]
                lv = 0 if cname in ("enc0", "dec0") else 1
                ru_t, c_t, uu_t = ru[(hf, lv)], ct[(hf, lv)], uu[(hf, lv)]
                box = {}

                def ph1():     # gate W-products (packed W1|2W2 matmuls)
                    tiles = []
                    for ci, (c0, cl) in enumerate(NCH):
                        dst = bp.tile([128, 1024], F16,
                                      tag=f"g{cname}{ci}_{hf}",
                                      name=f"g{cname}{ci}_{hf}")
                        # per-pair 1-bank stage tiles; evacs alternate
                        # scalar/vector so they run in parallel
                        for pr in range(2):
                            stg = psp.tile([128, 512], F32, tag="stage")
                            for j in range(2):
                                b = 2 * pr + j
                                nc.tensor.matmul(
                                    stg[0:cl, j * 256:(j + 1) * 256],
                                    xh_t[:, b * NB + c0: b * NB + c0 + cl],
                                    gw12,
                                    start=(j == 0), stop=(j == 1))
                            dsl = dst[0:cl, pr * 512:(pr + 1) * 512]
                            if pr == 0:
                                nc.vector.tensor_copy(dsl, stg[0:cl, :])
                            else:
                                nc.scalar.copy(dsl, stg[0:cl, :])
                        tiles.append(dst)
                    box["g"] = tiles

                def mk_ph2(pr):  # gate psum pair + sigmoid (+tail on pr1)
                    def ph2():
                        g = box["g"]
                        pg = pap.tile([128, 1024], F32, tag="pacc")
                        for j in range(2):
                            b = 2 * pr + j
                            off = j * 512
                            nc.tensor.matmul(pg[0:128, off:off + N],
                                             ga0,
                                             xh_t[:, b * NB:b * NB + N],
                                             start=True, stop=False)
                            for woff, srcs in ((0, s_t), (128, s2_t)):
                                for k, (c0, cl) in enumerate(NCH):
                                    nc.tensor.matmul(
                                        pg[0:128, off:off + N],
                                        g[k][0:cl,
                                             b * 256 + woff:b * 256 + woff + 128],
                                        srcs[k],
                                        start=False,
                                        stop=(woff == 128 and k == 2))
                        src = pg[0:128, :].rearrange(
                            "p (j n) -> p j n", n=512)[:, 0:2, 0:N]
                        dst = ru_t[:, :].rearrange(
                            "p (b n) -> p b n", n=NB)[:, 2 * pr:2 * pr + 2, 0:N]
                        nc.scalar.activation(dst, src, AFT.Sigmoid,
                                             bias=gb)
                        if pr == 1:
                            # u to partitions 0:64 (two-input DVE ops need
                            # equal base partitions) + r*h
                            nc.vector.tensor_copy(uu_t[0:64, :],
                                                  ru_t[64:128, :])
                            nc.vector.tensor_tensor(xr_t[0:64, :],
                                                    ru_t[0:64, :],
                                                    xh_t[0:64, :], ALU.mult)
                    return ph2

                def ph3():     # cand W-products (packed W1|2W2 matmuls)
                    tiles = []
                    for ci, (c0, cl) in enumerate(NCH):
                        stg = psp.tile([128, 512], F32, tag="stage")
                        for b in range(HB):
                            nc.tensor.matmul(
                                stg[0:cl, b * 128:(b + 1) * 128],
                                xr_t[:, b * NB + c0: b * NB + c0 + cl],
                                cw12,
                                start=(b == 0), stop=(b == HB - 1))
                        # de-interleave roles: b*128+{0:64 w1, 64:128 w2}
                        # -> p1 [cl, 4*64], p2 [cl, 4*64] (pair slices 2D)
                        sv = stg[0:cl, :].rearrange("p (b w f) -> p b w f",
                                                    w=2, f=64)
                        d1 = bp.tile([128, 256], F16, tag=f"c1{cname}{ci}_{hf}",
                                     name=f"c1{cname}{ci}_{hf}")
                        d2 = bp.tile([128, 256], F16, tag=f"c2{cname}{ci}_{hf}",
                                     name=f"c2{cname}{ci}_{hf}")
                        nc.scalar.copy(d1[0:cl, :], sv[:, :, 0, :])
                        nc.vector.tensor_copy(d2[0:cl, :], sv[:, :, 1, :])
                        tiles.append((d1, d2))
                    box["c"] = tiles

                def ph4():     # cand psum (both pairs) + tanh + GRU (+ mirror)
                    q = box["c"]
                    pc = pap.tile([128, 1024], F32, tag="pacc")
                    for pr in range(2):
                        off = pr * 512
                        first = True
                        for role, srcs in ((0, s_t), (1, s2_t)):
                            for k, (c0, cl) in enumerate(NCH):
                                lhs = q[k][role][0:cl,
                                                 pr * 128:(pr + 1) * 128]
                                nc.tensor.matmul(pc[0:128, off:off + N],
                                                 lhs, srcs[k],
                                                 start=first, stop=False)
                                first = False
                        for par in range(2):
                            b = 2 * pr + par
                            nc.tensor.matmul(
                                pc[par * 64:(par + 1) * 64, off:off + N],
                                ca0,
                                xr_t[:, b * NB:b * NB + N],
                                start=False, stop=(par == 1))
                    for par in range(2):
                        src = pc[par * 64:(par + 1) * 64, :].rearrange(
                            "p (j n) -> p j n", n=512)[:, 0:2, 0:N]
                        dst = c_t[0:64, :].rearrange(
                            "p (g q n) -> p g q n", q=2, n=NB)[:, :, par, 0:N]
                        nc.scalar.activation(dst, src, AFT.Tanh,
                                             bias=cb2[par])

                def ph5():     # GRU elementwise (no PE work)
                    # GRU: d = h - c -> xr; m = u*d; h' = c + m
                    nc.vector.tensor_tensor(xr_t[0:64, :], xh_t[0:64, :],
                                            c_t[0:64, :], ALU.subtract)
                    nc.vector.tensor_tensor(xr_t[0:64, :], uu_t[0:64, :],
                                            xr_t[0:64, :], ALU.mult)
                    nc.vector.tensor_tensor(xh_t[0:64, :], c_t[0:64, :],
                                            xr_t[0:64, :], ALU.add)

                def ph6():     # mirror h' into the next layer's x rows
                    if mirror_to is not None:
                        nc.vector.tensor_copy(
                            xh[(mirror_to, hf)][64:128, :], xh_t[0:64, :])
                        nc.scalar.copy(
                            xr[(mirror_to, hf)][64:128, :], xh_t[0:64, :])

                return [ph1, mk_ph2(0), mk_ph2(1), ph3, ph4, ph5, ph6]

            def interleave(*phase_lists):
                """Emit phase thunks round-robin: software-pipelines the
                independent cell streams so the PE queue never head-of-line
                blocks on one stream's evac/activation latency."""
                if os.environ.get("DCRNN_NO_PIPELINE"):
                    for pl in phase_lists:
                        for p in pl:
                            p()
                    return
                for i in range(max(len(p) for p in phase_lists)):
                    for pl in phase_lists:
                        if i < len(pl):
                            pl[i]()

            def proj_phase(hf, t):
                """Projection writes dec0 x-row directly, + output DMA +
                decoder feedback copy. Emitted AFTER dec1's ph4."""
                def ph():
                    xhd = xh[("dec0", hf)]
                    for pr in range(2):
                        pp = pap.tile([1, 1024], F32, tag="pacc")
                        for j in range(2):
                            b = 2 * pr + j
                            nc.tensor.matmul(
                                pp[0:1, j * 512:j * 512 + NB],
                                wt["pW"],
                                xh[("dec1", hf)][0:64, b * NB:(b + 1) * NB],
                                start=True, stop=True)
                        src = pp[0:1, :].rearrange(
                            "p (j n) -> p j n", n=512)[:, 0:2, 0:NB]
                        dst = xhd[64:65, :].rearrange(
                            "p (b n) -> p b n", n=NB)[:, 2 * pr:2 * pr + 2, :]
                        nc.scalar.activation(dst, src, AFT.Identity,
                                             bias=wt["pb1"])
                    ov = xhd[64:65, :].rearrange("p (b n) -> p b n", n=NB)
                    nc.sync.dma_start(out=out_d[t][:, hf * HB:(hf + 1) * HB, :],
                                      in_=ov[:, :, 0:N])
                    if t < nsteps_dec - 1:
                        nc.vector.tensor_copy(xr[("dec0", hf)][64:65, :],
                                              xhd[64:65, :])
                return ph

            # ---- encoder: enc1(t-1) pipelined against enc0(t) ----
            # staggered schedule: each phase sits several emission slots
            # after its producer so no engine queue tail stalls the PE.
            # A = enc0(t) (l0), B = enc1(t-1) (l1); mirror WAR ordering
            # requires B.ph3 before A.ph4.
            xr3 = x_in[:].rearrange("t d (g f) -> t d g f", g=2)
            prev_l1 = None
            for t in range(nsteps_enc):
                for hf in range(2):
                    nc.sync.dma_start(out=xh[("enc0", hf)][64:66, :],
                                      in_=xr3[t, :, hf, :])
                    nc.sync.dma_start(out=xr[("enc0", hf)][64:66, :],
                                      in_=xr3[t, :, hf, :])
                A = [cell_phases("enc0", hf, mirror_to="enc1")
                     for hf in range(2)]
                B = prev_l1
                if B is None:
                    interleave(*A)
                else:
                    # phases: 0 ph1, 1 ph2a, 2 ph2b, 3 ph3, 4 ph4,
                    # 5 ph5(GRU), 6 ph6(mirror). Hazard: B's matmul reads
                    # of the old mirror (through B.ph4) must precede
                    # A.ph6; A.ph5/ph6 sit mid-round so next round's ph1s
                    # never wait an engine queue tail.
                    for s, i in [(A, 0), (A, 1), (B, 0), (A, 2), (B, 1),
                                 (A, 3), (B, 2), (A, 4), (A, 5), (B, 3),
                                 (B, 4), (A, 6), (B, 5), (B, 6)]:
                        for hf in range(2):
                            s[hf][i]()
                prev_l1 = [cell_phases("enc1", hf) for hf in range(2)]

            # ---- last enc1 || copy encoder state to decoder ----
            interleave(*prev_l1)
            for hf in range(2):
                nc.vector.tensor_copy(xh[("dec0", hf)][0:64, :],
                                      xh[("enc0", hf)][0:64, :])
                nc.vector.tensor_copy(xh[("dec1", hf)][0:64, :],
                                      xh[("enc1", hf)][0:64, :])

            # ---- decoder (serial: dec0 -> dec1 -> proj feedback) ----
            # software pipeline: halves run the serial chain offset by one
            # cell position, so each half's chain joints are covered by
            # the other half's mid-cell PE work. hf0 at position m, hf1
            # at position m-1 (positions: 2t = dec0(t), 2t+1 = dec1+proj).
            def dec_pos(hf, p):
                t = p // 2
                if p % 2 == 0:
                    return cell_phases("dec0", hf, mirror_to="dec1")
                return cell_phases("dec1", hf) + [proj_phase(hf, t)]

            npos = 2 * nsteps_dec
            for m in range(npos + 1):
                lead = dec_pos(1, m - 1) if m >= 1 else []
                lag = dec_pos(0, m) if m < npos else []
                # lead runs 3 phases ahead of lag; lag's PE work is
                # enqueued first at each slot so lead's dependent matmuls
                # never head-of-line-block the PE queue
                for i in range(max(len(lead), len(lag) + 3)):
                    j = i - 3
                    if 0 <= j < len(lag):
                        lag[j]()
                    if i < len(lead):
                        lead[i]()

    nc.finalize()
    _BUILD_CACHE[key] = nc
    return nc


def _prep_inputs(inputs, support, weights):
    """Host-side prep. Returns (shared_map, per_core_x list)."""
    s32 = np.asarray(support, np.float32)
    s2_32 = s32 @ s32
    NWB = 4 * 576 + 6 * N + 1
    wb = np.zeros((128, NWB), np.float16)
    bb = np.zeros((128, 9), np.float32)
    for i, c in enumerate(CELLS):
        din = CELL_DIN[c]
        ga0, gw1, gw2 = _pad_w(weights[f"{c}_gate_W"], din, 2 * U)
        ca0, cw1, cw2 = _pad_w(weights[f"{c}_cand_W"], din, U)
        o = i * 576
        wb[:, o:o + 128] = ga0.astype(np.float16)
        wb[:, o + 128:o + 256] = gw1.astype(np.float16)
        wb[:, o + 256:o + 384] = gw2.astype(np.float16)
        wb[:, o + 384:o + 448] = ca0.astype(np.float16)
        wb[:, o + 448:o + 512] = cw1.astype(np.float16)
        wb[:, o + 512:o + 576] = cw2.astype(np.float16)
        bb[:, 2 * i] = weights[f"{c}_gate_b"]
        bb[0:64, 2 * i + 1] = weights[f"{c}_cand_b"]
        bb[64:128, 2 * i + 1] = weights[f"{c}_cand_b"]
    so = 4 * 576
    for ci, (c0, cl) in enumerate(NCH):
        wb[0:cl, so + ci * N: so + (ci + 1) * N] = \
            s32[c0:c0 + cl, :].astype(np.float16)
        wb[0:cl, so + (3 + ci) * N: so + (4 + ci) * N] = \
            s2_32[c0:c0 + cl, :].astype(np.float16)
    wb[0:64, NWB - 1] = np.asarray(weights["proj_W"]).reshape(-1).astype(
        np.float16)
    bb[0, 8] = float(np.asarray(weights["proj_b"]).reshape(-1)[0])
    shared = {"wb": wb, "bb": bb}

    # inputs (T, B, N*DIN) -> per-core (T, DIN, AF) with node padding
    x = np.asarray(inputs, np.float32).reshape(T, B, N, DIN)
    per_core = []
    for c in range(NCORES):
        xc = x[:, c * BL:(c + 1) * BL]                  # (T, BL, N, DIN)
        xp = np.zeros((T, DIN, BL, NB), np.float16)
        xp[:, :, :, 0:N] = xc.transpose(0, 3, 1, 2)
        per_core.append(xp.reshape(T, DIN, AF))
    return shared, per_core


def kernel(**inputs) -> np.ndarray:
    support = np.asarray(inputs["support"], np.float32)
    weights = {k: np.asarray(v, np.float32) for k, v in inputs.items()
               if k not in ("inputs", "support")}
    shared, per_core_x = _prep_inputs(inputs["inputs"], support, weights)

    nc = _build(T, HZ)
    if os.environ.get("DCRNN_TRACE"):
        _install_ntff_hook()
    in_maps = [dict(shared, x=per_core_x[c]) for c in range(NCORES)]
    res = run_bass_kernel_spmd(nc, in_maps, list(range(NCORES)),
                               trace=bool(os.environ.get("DCRNN_TRACE")))
    global LAST_RESULT
    LAST_RESULT = res
    if res.exec_time_ns is not None:
        print(f"HW exec time: {res.exec_time_ns} ns")
    outs = [res.results[c]["out"].reshape(HZ, BL, N) for c in range(NCORES)]
    return np.concatenate(outs, axis=1).astype(np.float32)


if __name__ == "__main__":
    sys.path.insert(0, "/root/problem")
    import reference
    ins = reference.setup_inputs()
    ins = {k: np.asarray(v) for k, v in ins.items()}
    exp = np.asarray(reference.reference(**ins))
    act = kernel(**ins)
    err = np.max(np.abs(act - exp)) / (np.abs(exp).max() + 1e-30)
    print("Relative error:", err)


# revision 35
# speedup vs baseline: 1.0866x; 1.0866x over previous
"""DCRNN (PEMS-BAY) Trainium2 Bass kernel, data-parallel over batch on 8 cores.

Transpose-free gconv via S^2 precompute, fp16 matmuls/states, fp32 psum
for the diffusion accumulations, fp16 psum staging for the W-products.

Layouts per core (local batch BL=8, split in 2 halves of HB=4):
  A1: [feature partitions, b*NB + n]   (state tiles XH/XR: rows 0:64 = h|rh,
      rows 64:64+din = x)
  B:  [node-chunk partitions (128/128/69), b*F + f]  (W-product tiles)
gconv:  pre = X@A0 + S@(X@W1) + S^2@(X@(2*W2))      [A0 = W0 - W2]
  ph1/ph3 W-products: ONE matmul per (b, chunk) with packed rhs [W1 | 2*W2]
  into fp16 psum, ONE evac copy per chunk into a packed B-tile.
  ph2/ph4: diffusion accumulation per batch in fp32 psum 2-bank pair tiles
  (P0 = a0 start, then S@P1 + S2@P2 with S/S2 as rhs, lhsT = B-tile slices),
  activations applied per PAIR via 3-dim APs spanning both banks.
GRU elementwise merged to [64, HAF] half-wide ops; mirror copies via DMA.
"""
import sys
import os
import numpy as np

sys.path.insert(0, "/opt/trn_rl_repo")

import concourse.bass as bass  # noqa: E402
import concourse.mybir as mybir  # noqa: E402
import concourse.tile as tile  # noqa: E402
from concourse import bacc  # noqa: E402
from concourse.bass_utils import run_bass_kernel_spmd  # noqa: E402

# problem constants
N = 325
B = 64
T = 12
HZ = 12
U = 64
DIN = 2
DOUT = 1
NCORES = 8
BL = B // NCORES          # 8 local batch
NB = 328                  # padded node stride per batch (16B aligned)
AF = BL * NB              # 2624 A-layout free width
NCH = [(0, 128), (128, 128), (256, 69)]   # node chunks (offset, len)
HB = BL // 2              # 4 batches per half
HAF = HB * NB             # 1312 A cols per half

F32 = mybir.dt.float32
F16 = mybir.dt.float16
AFT = mybir.ActivationFunctionType
ALU = mybir.AluOpType

CELLS = ["enc0", "enc1", "dec0", "dec1"]
CELL_DIN = {"enc0": DIN, "enc1": U, "dec0": DOUT, "dec1": U}

_BUILD_CACHE = {}
LAST_RESULT = None


def _install_ntff_hook():
    """Register the axon NTFF profiling hook if the image lacks antenv.axon_hooks."""
    import types
    import antenv
    if getattr(antenv, "axon_hooks", None) is not None:
        return
    m = types.ModuleType("antenv.axon_hooks")
    state = {"h": None}
    m.set_axon_ntff_profile_hook = lambda h: state.__setitem__("h", h)
    m.get_axon_ntff_profile_hook = lambda: state["h"]
    sys.modules["antenv.axon_hooks"] = m
    antenv.axon_hooks = m
    try:
        from trn_agent_boot.trn_boot import _ntff_profile_via_ctypes
        hook = _ntff_profile_via_ctypes("/opt/axon/libaxon_pjrt.so")
        if hook is not None:
            m.set_axon_ntff_profile_hook(hook)
    except Exception:
        pass


def _pad_w(w, din, fout):
    """(3F, fout) -> three [128, fout] fp32 mats A0, W1, 2*W2.

    Padded row map: rows 0:64 <- h/rh features (orig rows din:F),
    rows 64:64+din <- x features (orig rows 0:din). Others zero.
    """
    f = din + U
    w0, w1, w2 = w[0:f], w[f:2 * f], w[2 * f:3 * f]

    def pad(m):
        p = np.zeros((128, fout), np.float32)
        p[0:64] = m[din:f]
        p[64:64 + din] = m[0:din]
        return p

    return pad(w0 - w2), pad(w1), pad(2.0 * w2)


def _build(nsteps_enc, nsteps_dec):
    key = (nsteps_enc, nsteps_dec)
    if key in _BUILD_CACHE:
        return _BUILD_CACHE[key]

    nc = bacc.Bacc()
    # ---- DRAM params ----
    # all fp16 weights live in one blob (single DMA at startup):
    # per cell [gA0 128 | gW12 256 | cA0 64 | cW12 128] = 576 cols,
    # then s chunks 3x325, s2 chunks 3x325, pW 1 col.
    NWB = 4 * 576 + 6 * N + 1
    x_in = nc.declare_dram_parameter("x", [T, DIN, AF], F16, isOutput=False)
    wb_in = nc.declare_dram_parameter("wb", [128, NWB], F16, isOutput=False)
    bb_in = nc.declare_dram_parameter("bb", [128, 9], F32, isOutput=False)
    out_d = nc.declare_dram_parameter("out", [HZ, 1, BL, N], F16, isOutput=True)

    with tile.TileContext(nc) as tc:
        with tc.tile_pool(name="const", bufs=1) as cp, \
             tc.tile_pool(name="state", bufs=1) as st, \
             tc.tile_pool(name="bprod", bufs=1) as bp, \
             tc.tile_pool(name="pstage", bufs=4, space="PSUM") as psp, \
             tc.tile_pool(name="pacc", bufs=2, space="PSUM") as pap:

            # ---- constants to SBUF (one DMA each for wb/bb) ----
            wbt = cp.tile([128, NWB], F16, tag="wb", name="wb")
            bbt = cp.tile([128, 9], F32, tag="bb", name="bb")
            nc.sync.dma_start(out=wbt, in_=wb_in[:])
            nc.sync.dma_start(out=bbt, in_=bb_in[:])
            wt = {}
            for i, c in enumerate(CELLS):
                o = i * 576
                wt[f"{c}_gA0"] = wbt[0:128, o:o + 128]
                wt[f"{c}_gW12"] = wbt[0:128, o + 128:o + 384]
                wt[f"{c}_cA0"] = wbt[0:128, o + 384:o + 448]
                wt[f"{c}_cW12"] = wbt[0:128, o + 448:o + 576]
                wt[f"{c}_gb"] = bbt[0:128, 2 * i:2 * i + 1]
                wt[f"{c}_cb2"] = [bbt[0:64, 2 * i + 1:2 * i + 2],
                                  bbt[64:128, 2 * i + 1:2 * i + 2]]
            so = 4 * 576
            s_t = [wbt[0:cl, so + ci * N: so + (ci + 1) * N]
                   for ci, (c0, cl) in enumerate(NCH)]
            s2_t = [wbt[0:cl, so + (3 + ci) * N: so + (4 + ci) * N]
                    for ci, (c0, cl) in enumerate(NCH)]
            wt["pW"] = wbt[0:64, NWB - 1:NWB]
            wt["pb1"] = bbt[0:1, 8:9]

            # ---- state tiles ----
            xh, xr = {}, {}
            ru, ct, uu = {}, {}, {}
            for hf in range(2):
                for c in CELLS:
                    xh[(c, hf)] = st.tile([128, HAF], F16, tag=f"xh_{c}_{hf}",
                                          name=f"xh_{c}_{hf}")
                    xr[(c, hf)] = st.tile([128, HAF], F16, tag=f"xr_{c}_{hf}",
                                          name=f"xr_{c}_{hf}")
                for lv in range(2):
                    ru[(hf, lv)] = st.tile([128, HAF], F16, tag=f"ru{hf}{lv}",
                                           name=f"ru{hf}{lv}")
                    ct[(hf, lv)] = st.tile([64, HAF], F16, tag=f"ct{hf}{lv}",
                                           name=f"ct{hf}{lv}")
                    uu[(hf, lv)] = st.tile([64, HAF], F16, tag=f"uu{hf}{lv}",
                                           name=f"uu{hf}{lv}")

            # Only xh/xr need zeroing (their unwritten rows feed matmul
            # contractions where a NaN would poison the psum even against
            # zero weights). ru/ct/uu pad garbage stays confined to pad
            # columns which nothing downstream reads. Ordered by first use.
            ms = []
            for c in CELLS:
                for hf in range(2):
                    ms += [xh[(c, hf)], xr[(c, hf)]]
            for tl in ms:
                nc.vector.memset(tl[:, :], 0.0)

            def cell_phases(cname, hf, mirror_to=None):
                """One DCGRU cell on one batch-half, as 4 phase thunks.

                mirror_to: layer name whose x rows receive h' per batch.
                """
                xh_t = xh[(cname, hf)]
                xr_t = xr[(cname, hf)]
                gb = wt[f"{cname}_gb"]
                cb2 = wt[f"{cname}_cb2"]
                ga0 = wt[f"{cname}_gA0"]
                gw12 = wt[f"{cname}_gW12"]
                ca0 = wt[f"{cname}_cA0"]
                cw12 = wt[f"{cname}_cW12"]
                lv = 0 if cname in ("enc0", "dec0") else 1
                ru_t, c_t, uu_t = ru[(hf, lv)], ct[(hf, lv)], uu[(hf, lv)]
                box = {}

                def ph1():     # gate W-products (packed W1|2W2 matmuls)
                    tiles = []
                    for ci, (c0, cl) in enumerate(NCH):
                        dst = bp.tile([128, 1024], F16,
                                      tag=f"g{cname}{ci}_{hf}",
                                      name=f"g{cname}{ci}_{hf}")
                        # per-pair 1-bank stage tiles; evacs alternate
                        # scalar/vector so they run in parallel
                        for pr in range(2):
                            stg = psp.tile([128, 512], F32, tag="stage")
                            for j in range(2):
                                b = 2 * pr + j
                                nc.tensor.matmul(
                                    stg[0:cl, j * 256:(j + 1) * 256],
                                    xh_t[:, b * NB + c0: b * NB + c0 + cl],
                                    gw12,
                                    start=(j == 0), stop=(j == 1))
                            dsl = dst[0:cl, pr * 512:(pr + 1) * 512]
                            if pr == 0:
                                nc.vector.tensor_copy(dsl, stg[0:cl, :])
                            else:
                                nc.scalar.copy(dsl, stg[0:cl, :])
                        tiles.append(dst)
                    box["g"] = tiles

                def ph2():     # gate psum pairs + sigmoid + u + rh
                    g = box["g"]
                    for pr in range(2):
                        pg = pap.tile([128, 1024], F32, tag="pacc")
                        for j in range(2):
                            b = 2 * pr + j
                            off = j * 512
                            nc.tensor.matmul(pg[0:128, off:off + N],
                                             ga0,
                                             xh_t[:, b * NB:b * NB + N],
                                             start=True, stop=False)
                            for woff, srcs in ((0, s_t), (128, s2_t)):
                                for k, (c0, cl) in enumerate(NCH):
                                    nc.tensor.matmul(
                                        pg[0:128, off:off + N],
                                        g[k][0:cl,
                                             b * 256 + woff:b * 256 + woff + 128],
                                        srcs[k],
                                        start=False,
                                        stop=(woff == 128 and k == 2))
                        src = pg[0:128, :].rearrange(
                            "p (j n) -> p j n", n=512)[:, 0:2, 0:N]
                        dst = ru_t[:, :].rearrange(
                            "p (b n) -> p b n", n=NB)[:, 2 * pr:2 * pr + 2, 0:N]
                        nc.scalar.activation(dst, src, AFT.Sigmoid,
                                             bias=gb)
                    # u to partitions 0:64 (two-input DVE ops need equal
                    # base partitions) + r*h
                    nc.vector.tensor_copy(uu_t[0:64, :], ru_t[64:128, :])
                    nc.vector.tensor_tensor(xr_t[0:64, :], ru_t[0:64, :],
                                            xh_t[0:64, :], ALU.mult)

                def ph3():     # cand W-products (packed W1|2W2 matmuls)
                    tiles = []
                    for ci, (c0, cl) in enumerate(NCH):
                        stg = psp.tile([128, 512], F32, tag="stage")
                        for b in range(HB):
                            nc.tensor.matmul(
                                stg[0:cl, b * 128:(b + 1) * 128],
                                xr_t[:, b * NB + c0: b * NB + c0 + cl],
                                cw12,
                                start=(b == 0), stop=(b == HB - 1))
                        # de-interleave roles: b*128+{0:64 w1, 64:128 w2}
                        # -> p1 [cl, 4*64], p2 [cl, 4*64] (pair slices 2D)
                        sv = stg[0:cl, :].rearrange("p (b w f) -> p b w f",
                                                    w=2, f=64)
                        d1 = bp.tile([128, 256], F16, tag=f"c1{cname}{ci}_{hf}",
                                     name=f"c1{cname}{ci}_{hf}")
                        d2 = bp.tile([128, 256], F16, tag=f"c2{cname}{ci}_{hf}",
                                     name=f"c2{cname}{ci}_{hf}")
                        nc.scalar.copy(d1[0:cl, :], sv[:, :, 0, :])
                        nc.vector.tensor_copy(d2[0:cl, :], sv[:, :, 1, :])
                        tiles.append((d1, d2))
                    box["c"] = tiles

                def ph4():     # cand psum (both pairs) + tanh + GRU (+ mirror)
                    q = box["c"]
                    pc = pap.tile([128, 1024], F32, tag="pacc")
                    for pr in range(2):
                        off = pr * 512
                        first = True
                        for role, srcs in ((0, s_t), (1, s2_t)):
                            for k, (c0, cl) in enumerate(NCH):
                                lhs = q[k][role][0:cl,
                                                 pr * 128:(pr + 1) * 128]
                                nc.tensor.matmul(pc[0:128, off:off + N],
                                                 lhs, srcs[k],
                                                 start=first, stop=False)
                                first = False
                        for par in range(2):
                            b = 2 * pr + par
                            nc.tensor.matmul(
                                pc[par * 64:(par + 1) * 64, off:off + N],
                                ca0,
                                xr_t[:, b * NB:b * NB + N],
                                start=False, stop=(par == 1))
                    for par in range(2):
                        src = pc[par * 64:(par + 1) * 64, :].rearrange(
                            "p (j n) -> p j n", n=512)[:, 0:2, 0:N]
                        dst = c_t[0:64, :].rearrange(
                            "p (g q n) -> p g q n", q=2, n=NB)[:, :, par, 0:N]
                        nc.scalar.activation(dst, src, AFT.Tanh,
                                             bias=cb2[par])

                def ph5():     # GRU elementwise + mirror (no PE work)
                    # GRU: d = h - c -> xr; m = u*d; h' = c + m
                    nc.vector.tensor_tensor(xr_t[0:64, :], xh_t[0:64, :],
                                            c_t[0:64, :], ALU.subtract)
                    nc.vector.tensor_tensor(xr_t[0:64, :], uu_t[0:64, :],
                                            xr_t[0:64, :], ALU.mult)
                    nc.vector.tensor_tensor(xh_t[0:64, :], c_t[0:64, :],
                                            xr_t[0:64, :], ALU.add)
                    if mirror_to is not None:
                        nc.vector.tensor_copy(
                            xh[(mirror_to, hf)][64:128, :], xh_t[0:64, :])
                        nc.scalar.copy(
                            xr[(mirror_to, hf)][64:128, :], xh_t[0:64, :])

                return [ph1, ph2, ph3, ph4, ph5]

            def interleave(*phase_lists):
                """Emit phase thunks round-robin: software-pipelines the
                independent cell streams so the PE queue never head-of-line
                blocks on one stream's evac/activation latency."""
                if os.environ.get("DCRNN_NO_PIPELINE"):
                    for pl in phase_lists:
                        for p in pl:
                            p()
                    return
                for i in range(max(len(p) for p in phase_lists)):
                    for pl in phase_lists:
                        if i < len(pl):
                            pl[i]()

            def proj_phase(hf, t):
                """Projection writes dec0 x-row directly, + output DMA +
                decoder feedback copy. Emitted AFTER dec1's ph4."""
                def ph():
                    xhd = xh[("dec0", hf)]
                    for pr in range(2):
                        pp = pap.tile([1, 1024], F32, tag="pacc")
                        for j in range(2):
                            b = 2 * pr + j
                            nc.tensor.matmul(
                                pp[0:1, j * 512:j * 512 + NB],
                                wt["pW"],
                                xh[("dec1", hf)][0:64, b * NB:(b + 1) * NB],
                                start=True, stop=True)
                        src = pp[0:1, :].rearrange(
                            "p (j n) -> p j n", n=512)[:, 0:2, 0:NB]
                        dst = xhd[64:65, :].rearrange(
                            "p (b n) -> p b n", n=NB)[:, 2 * pr:2 * pr + 2, :]
                        nc.scalar.activation(dst, src, AFT.Identity,
                                             bias=wt["pb1"])
                    ov = xhd[64:65, :].rearrange("p (b n) -> p b n", n=NB)
                    nc.sync.dma_start(out=out_d[t][:, hf * HB:(hf + 1) * HB, :],
                                      in_=ov[:, :, 0:N])
                    if t < nsteps_dec - 1:
                        nc.vector.tensor_copy(xr[("dec0", hf)][64:65, :],
                                              xhd[64:65, :])
                return ph

            # ---- encoder: enc1(t-1) pipelined against enc0(t) ----
            # staggered schedule: each phase sits several emission slots
            # after its producer so no engine queue tail stalls the PE.
            # A = enc0(t) (l0), B = enc1(t-1) (l1); mirror WAR ordering
            # requires B.ph3 before A.ph4.
            xr3 = x_in[:].rearrange("t d (g f) -> t d g f", g=2)
            prev_l1 = None
            for t in range(nsteps_enc):
                for hf in range(2):
                    nc.sync.dma_start(out=xh[("enc0", hf)][64:66, :],
                                      in_=xr3[t, :, hf, :])
                    nc.sync.dma_start(out=xr[("enc0", hf)][64:66, :],
                                      in_=xr3[t, :, hf, :])
                A = [cell_phases("enc0", hf, mirror_to="enc1")
                     for hf in range(2)]
                B = prev_l1
                if B is None:
                    interleave(*A)
                else:
                    # B.ph3/ph4 must precede A.ph5 (A's mirror overwrites
                    # the x rows B's cand matmuls read); A.ph5 sits mid-
                    # round so next round's ph1s never wait a queue tail.
                    for s, i in [(A, 0), (A, 1), (B, 0), (A, 2), (B, 1),
                                 (B, 2), (A, 3), (B, 3), (A, 4), (B, 4)]:
                        for hf in range(2):
                            s[hf][i]()
                prev_l1 = [cell_phases("enc1", hf) for hf in range(2)]

            # ---- last enc1 || copy encoder state to decoder ----
            interleave(*prev_l1)
            for hf in range(2):
                nc.vector.tensor_copy(xh[("dec0", hf)][0:64, :],
                                      xh[("enc0", hf)][0:64, :])
                nc.vector.tensor_copy(xh[("dec1", hf)][0:64, :],
                                      xh[("enc1", hf)][0:64, :])

            # ---- decoder (serial: dec0 -> dec1 -> proj feedback) ----
            # halves staggered so each dependency has PE work in between
            for t in range(nsteps_dec):
                d0 = [cell_phases("dec0", hf, mirror_to="dec1")
                      for hf in range(2)]
                d1 = [cell_phases("dec1", hf) for hf in range(2)]
                pj = [proj_phase(hf, t) for hf in range(2)]
                for s, h, i in [(d0, 0, 0), (d0, 0, 1), (d0, 1, 0),
                                (d0, 0, 2), (d0, 1, 1), (d0, 0, 3),
                                (d0, 1, 2), (d0, 0, 4), (d0, 1, 3),
                                (d0, 1, 4),
                                (d1, 0, 0), (d1, 0, 1), (d1, 1, 0),
                                (d1, 0, 2), (d1, 1, 1), (d1, 0, 3),
                                (d1, 1, 2), (d1, 0, 4), (d1, 1, 3),
                                (pj, 0, None), (d1, 1, 4), (pj, 1, None)]:
                    if i is None:
                        s[h]()
                    else:
                        s[h][i]()

    nc.finalize()
    _BUILD_CACHE[key] = nc
    return nc


def _prep_inputs(inputs, support, weights):
    """Host-side prep. Returns (shared_map, per_core_x list)."""
    s32 = np.asarray(support, np.float32)
    s2_32 = s32 @ s32
    NWB = 4 * 576 + 6 * N + 1
    wb = np.zeros((128, NWB), np.float16)
    bb = np.zeros((128, 9), np.float32)
    for i, c in enumerate(CELLS):
        din = CELL_DIN[c]
        ga0, gw1, gw2 = _pad_w(weights[f"{c}_gate_W"], din, 2 * U)
        ca0, cw1, cw2 = _pad_w(weights[f"{c}_cand_W"], din, U)
        o = i * 576
        wb[:, o:o + 128] = ga0.astype(np.float16)
        wb[:, o + 128:o + 256] = gw1.astype(np.float16)
        wb[:, o + 256:o + 384] = gw2.astype(np.float16)
        wb[:, o + 384:o + 448] = ca0.astype(np.float16)
        wb[:, o + 448:o + 512] = cw1.astype(np.float16)
        wb[:, o + 512:o + 576] = cw2.astype(np.float16)
        bb[:, 2 * i] = weights[f"{c}_gate_b"]
        bb[0:64, 2 * i + 1] = weights[f"{c}_cand_b"]
        bb[64:128, 2 * i + 1] = weights[f"{c}_cand_b"]
    so = 4 * 576
    for ci, (c0, cl) in enumerate(NCH):
        wb[0:cl, so + ci * N: so + (ci + 1) * N] = \
            s32[c0:c0 + cl, :].astype(np.float16)
        wb[0:cl, so + (3 + ci) * N: so + (4 + ci) * N] = \
            s2_32[c0:c0 + cl, :].astype(np.float16)
    wb[0:64, NWB - 1] = np.asarray(weights["proj_W"]).reshape(-1).astype(
        np.float16)
    bb[0, 8] = float(np.asarray(weights["proj_b"]).reshape(-1)[0])
    shared = {"wb": wb, "bb": bb}

    # inputs (T, B, N*DIN) -> per-core (T, DIN, AF) with node padding
    x = np.asarray(inputs, np.float32).reshape(T, B, N, DIN)
    per_core = []
    for c in range(NCORES):
        xc = x[:, c * BL:(c + 1) * BL]                  # (T, BL, N, DIN)
        xp = np.zeros((T, DIN, BL, NB), np.float16)
        xp[:, :, :, 0:N] = xc.transpose(0, 3, 1, 2)
        per_core.append(xp.reshape(T, DIN, AF))
    return shared, per_core


def kernel(**inputs) -> np.ndarray:
    support = np.asarray(inputs["support"], np.float32)
    weights = {k: np.asarray(v, np.float32) for k, v in inputs.items()
               if k not in ("inputs", "support")}
    shared, per_core_x = _prep_inputs(inputs["inputs"], support, weights)

    nc = _build(T, HZ)
    if os.environ.get("DCRNN_TRACE"):
        _install_ntff_hook()
    in_maps = [dict(shared, x=per_core_x[c]) for c in range(NCORES)]
    res = run_bass_kernel_spmd(nc, in_maps, list(range(NCORES)),
                               trace=bool(os.environ.get("DCRNN_TRACE")))
    global LAST_RESULT
    LAST_RESULT = res
    if res.exec_time_ns is not None:
        print(f"HW exec time: {res.exec_time_ns} ns")
    outs = [res.results[c]["out"].reshape(HZ, BL, N) for c in range(NCORES)]
    return np.concatenate(outs, axis=1).astype(np.float32)


if __name__ == "__main__":
    sys.path.insert(0, "/root/problem")
    import reference
    ins = reference.setup_inputs()
    ins = {k: np.asarray(v) for k, v in ins.items()}
    exp = np.asarray(reference.reference(**ins))
    act = kernel(**ins)
    err = np.max(np.abs(act - exp)) / (np.abs(exp).max() + 1e-30)
    print("Relative error:", err)
